# revision 4
# baseline (speedup 1.0000x reference)
"""Causal multi-head self-attention with RoPE on 8 TRN2 NeuronCores.

Sharding: data-parallel over batch (4) x tensor-parallel over heads (16 -> 2
groups of 8).  Core c handles batch c//2, head group c%2.  Each core computes
its 8 heads' attention and a partial O-projection (512 of the 1024 contraction
dims); the host sums the two partials per batch element.

v2: all per-core inputs packed into ONE bf16 dram blob; bf16 output; head
PAIRS processed together in attention -- the two 64-row score matmuls run
concurrently on disjoint PE row-groups into one 2-bank PSUM tile, a single
(free-strided) exp covers both heads, the intra-tile causal mask is one
[128,2x128] multiply, and the AV matmuls are trimmed to the valid column
span so masked prefixes never reach PSUM.
"""

import os
import sys

import numpy as np

if "/opt/trn_rl_repo" not in sys.path:
    sys.path.insert(0, "/opt/trn_rl_repo")

D_MODEL = 1024
NUM_HEADS = 16
THETA = 10000.0
B, S = 4, 2048
DK = 64
HALF = DK // 2
P = 128
N_CORES = 8
HPC = 8                 # heads per core
DOUT = HPC * DK         # 512 per-core projected dims
KT = D_MODEL // P       # 8 contraction tiles
NSEQ = S // P           # 16 seq tiles of 128
NQB = S // 512          # 4 query blocks of 512
SCALE = 1.0 / np.sqrt(DK)

# packed blob layout (bf16 element offsets)
_SEGS = [
    ("xT", D_MODEL * S),           # [1024, 2048]
    ("wq", D_MODEL * DOUT),        # [1024, 512]
    ("wk", D_MODEL * DOUT),
    ("wv", D_MODEL * DOUT),
    ("cos", S * DK),               # [2048, 64]
    ("sin", S * DK),
    ("tri", P * P),                # [128, 128] lower-tri 0/1
    ("ident", P * P),              # [128, 128]
    ("wo", DOUT * D_MODEL),        # [512, 1024]
]
_OFF = {}
_t = 0
for _nm, _sz in _SEGS:
    _OFF[_nm] = _t
    _t += _sz
BLOB_N = _t

_CACHE = {}


def _build(reps=1):
    """Build + compile the per-core Bass module (same program on all cores).

    reps > 1 repeats the whole body (input DMAs + compute + stores) that many
    times in one NEFF; used by the benchmark to measure marginal exec time.
    """
    import concourse.bass as bass
    import concourse.bacc as bacc
    import concourse.tile as tile
    import concourse.mybir as mybir
    from contextlib import ExitStack

    f32 = mybir.dt.float32
    bf16 = mybir.dt.bfloat16
    Exp = mybir.ActivationFunctionType.Exp

    nc = bacc.Bacc("TRN2", target_bir_lowering=False, debug=False,
                   enable_asserts=False, num_devices=N_CORES)

    blob = nc.dram_tensor("blob", [BLOB_N], bf16, kind="ExternalInput")
    out = nc.dram_tensor("out", [S, D_MODEL], bf16, kind="ExternalOutput")
    blob_t = blob[0:1].tensor

    def brows(name, row0, nrows, rowlen):
        # [nrows, rowlen] 2-D window into the flat blob segment `name`
        return bass.AP(tensor=blob_t, offset=_OFF[name] + row0 * rowlen,
                       ap=[[rowlen, nrows], [1, rowlen]])

    def rep8(ap):
        # replicate a [128, 64] tile 8x along free dim -> logical [128, 512]
        return bass.AP(tensor=ap.tensor, offset=ap.offset,
                       ap=[ap.ap[0], [0, HPC], [1, DK]])

    def pairswap(ap):
        # free-dim pair swap of a [128, 512] tile: (0,1,2,3,..)->(1,0,3,2,..)
        return bass.AP(tensor=ap.tensor, offset=ap.offset + 1,
                       ap=[ap.ap[0], [2, 256], [-1, 2]])

    def two_slabs(ap, off, width, slab=512):
        # [128, 2, width] view: cols [off, off+width) of both 512-col halves
        return bass.AP(tensor=ap.tensor, offset=ap.offset + off,
                       ap=[ap.ap[0], [slab, 2], [1, width]])

    def bcast2(ap, width):
        # [128, 2, width] view of a [128, width] tile (free-stride-0 repeat)
        return bass.AP(tensor=ap.tensor, offset=ap.offset,
                       ap=[ap.ap[0], [0, 2], [1, width]])

    with tile.TileContext(nc) as tc, ExitStack() as top:
        persist = top.enter_context(tc.tile_pool(name="persist", bufs=1))
        # psum (8 banks): proj/oproj 1, transpose 1, scores 2x2, av 2
        mm_ps = top.enter_context(tc.tile_pool(name="mm_ps", bufs=1, space="PSUM"))
        tr_ps = top.enter_context(tc.tile_pool(name="tr_ps", bufs=1, space="PSUM"))
        sc_ps = top.enter_context(tc.tile_pool(name="sc_ps", bufs=2, space="PSUM"))
        av_ps = top.enter_context(tc.tile_pool(name="av_ps", bufs=1, space="PSUM"))
        ropet = top.enter_context(tc.tile_pool(name="ropet", bufs=2))
        natp = top.enter_context(tc.tile_pool(name="natp", bufs=4))
        ptp = top.enter_context(tc.tile_pool(name="ptp", bufs=4))
        rcpp = top.enter_context(tc.tile_pool(name="rcpp", bufs=2))
        rmatp = top.enter_context(tc.tile_pool(name="rmatp", bufs=2))
        ostg = top.enter_context(tc.tile_pool(name="ostg", bufs=3))

        def emit_body():
            # ---- persistent SBUF arrays ----
            x_sb = [persist.tile([P, S], bf16, tag=f"x{k}", name=f"x{k}")
                    for k in range(KT)]
            w_sb = {nm: [persist.tile([P, DOUT], bf16, tag=f"{nm}{k}",
                                      name=f"{nm}{k}") for k in range(KT)]
                    for nm in ("wq", "wk", "wv")}
            wo_sb = [persist.tile([P, D_MODEL], bf16, tag=f"wo{k}",
                                  name=f"wo{k}") for k in range(DOUT // P)]
            cos_sb = [persist.tile([P, DK], bf16, tag=f"cos{m}",
                                   name=f"cos{m}") for m in range(NSEQ)]
            sin_sb = [persist.tile([P, DK], bf16, tag=f"sin{m}",
                                   name=f"sin{m}") for m in range(NSEQ)]
            tri_sb = persist.tile([P, P], bf16, tag="tri", name="tri")
            id_sb = persist.tile([P, P], bf16, tag="ident", name="ident")

            for k in range(KT):
                nc.sync.dma_start(out=x_sb[k], in_=brows("xT", k * P, P, S))
                nc.sync.dma_start(out=w_sb["wq"][k],
                                  in_=brows("wq", k * P, P, DOUT))
            for m in range(8):
                nc.sync.dma_start(out=cos_sb[m], in_=brows("cos", m * P, P, DK))
                nc.sync.dma_start(out=sin_sb[m], in_=brows("sin", m * P, P, DK))
            nc.sync.dma_start(out=id_sb, in_=brows("ident", 0, P, P))
            for k in range(KT):
                nc.sync.dma_start(out=w_sb["wk"][k],
                                  in_=brows("wk", k * P, P, DOUT))
            for m in range(8, NSEQ):
                nc.sync.dma_start(out=cos_sb[m], in_=brows("cos", m * P, P, DK))
                nc.sync.dma_start(out=sin_sb[m], in_=brows("sin", m * P, P, DK))
            for k in range(KT):
                nc.sync.dma_start(out=w_sb["wv"][k],
                                  in_=brows("wv", k * P, P, DOUT))
            nc.sync.dma_start(out=tri_sb, in_=brows("tri", 0, P, P))
            for k in range(DOUT // P):
                nc.sync.dma_start(out=wo_sb[k],
                                  in_=brows("wo", k * P, P, D_MODEL))

            # outputs of phase A: qt/kt as single wide tensors
            # [128, 4*2048]: db-th 2048-col segment = transposed head-pair db
            qt_sb = persist.tile([P, 4 * S], bf16, tag="qt", name="qt")
            kt_sb = persist.tile([P, 4 * S], bf16, tag="kt", name="kt")
            v_sb = [persist.tile([P, HPC * (DK + 1)], bf16, tag=f"v{t}",
                                 name=f"v{t}") for t in range(NSEQ)]
            ot_sb = [persist.tile([P, S], bf16, tag=f"ot{d}", name=f"ot{d}")
                     for d in range(4)]

            def drain(dst, ptr, pm):
                # one strided copy: trt [128, 4x128] -> 4 segments of dst
                dap = bass.AP(tensor=dst.tensor,
                              offset=dst.offset + pm * P,
                              ap=[dst.ap[0], [S, 4], [1, P]])
                nc.vector.tensor_copy(dap, ptr)

            def proj_group(g):
                """Q/K/V projections + rope + transpose, m in [4g, 4g+4)."""
                for nm, dst in (("wq", qt_sb), ("wk", kt_sb)):
                    pend = None          # lag-1 transpose drain: (trt, m)
                    for m in range(4 * g, 4 * g + 4):
                        ps = mm_ps.tile([P, DOUT], f32, tag="mm", name="mm")
                        for k in range(KT):
                            nc.tensor.matmul(ps, x_sb[k][:, m * P:(m + 1) * P],
                                             w_sb[nm][k], start=(k == 0),
                                             stop=(k == KT - 1))
                        t1 = ropet.tile([P, DOUT], f32, tag="rt1", name="rt1")
                        t2 = ropet.tile([P, DOUT], f32, tag="rt2", name="rt2")
                        nc.vector.tensor_mul(t1, ps, rep8(cos_sb[m]))
                        nc.vector.tensor_mul(t2, pairswap(ps), rep8(sin_sb[m]))
                        nat = natp.tile([P, DOUT], bf16, tag="nat", name="nat")
                        nc.vector.tensor_add(nat, t1, t2)
                        if pend is not None:
                            drain(dst, *pend)
                        trt = tr_ps.tile([P, 512], bf16, tag="tr", name="trt")
                        for d in range(4):
                            nc.tensor.transpose(trt[:, d * P:(d + 1) * P],
                                                nat[:, d * P:(d + 1) * P],
                                                id_sb)
                        pend = (trt, m)
                    drain(dst, *pend)
                for m in range(4 * g, 4 * g + 4):
                    ps = mm_ps.tile([P, DOUT], f32, tag="mm", name="mm")
                    for k in range(KT):
                        nc.tensor.matmul(ps, x_sb[k][:, m * P:(m + 1) * P],
                                         w_sb["wv"][k], start=(k == 0),
                                         stop=(k == KT - 1))
                    vt = v_sb[m]
                    ones_ap = bass.AP(tensor=vt.tensor, offset=vt.offset + DK,
                                      ap=[vt.ap[0], [DK + 1, HPC]])
                    nc.gpsimd.memset(ones_ap, 1.0)
                    vcols = bass.AP(tensor=vt.tensor, offset=vt.offset,
                                    ap=[vt.ap[0], [DK + 1, HPC], [1, DK]])
                    nc.scalar.copy(vcols, ps)

            def attn_group(g):
                """Attention for query block qb=g, head pairs (2db, 2db+1).

                Per kv tile t, the two heads' score matmuls target the two
                512-col halves of one [128,1024] psum tile and run on
                disjoint PE row groups (base partitions 0/64).  Diagonal
                tiles (v = t-4g >= 0) are trimmed to q-cols >= 128v; one
                [128,2x128] multiply with the lower-tri mask clears the
                intra-tile upper triangle, and AV only reads valid columns.
                """
                cols = slice(g * 512, (g + 1) * 512)
                nt = 4 * g + 4
                for db in range(4):
                    seg = db * S
                    av = av_ps.tile([DK + 1, 1024], f32, tag="av", name="av")
                    for t in range(nt):
                        v = t - 4 * g
                        c0 = 128 * v if v >= 0 else 0
                        w = 512 - c0
                        sc = sc_ps.tile([P, 1024], f32, tag="sc", name="sc")
                        for hh in range(2):
                            po = hh * DK
                            nc.tensor.matmul(
                                sc[:, hh * 512 + c0:(hh + 1) * 512],
                                kt_sb[po:po + DK,
                                      seg + t * P:seg + (t + 1) * P],
                                qt_sb[po:po + DK,
                                      seg + g * 512 + c0:seg + (g + 1) * 512],
                                start=True, stop=True)
                        pt = ptp.tile([P, 1024], bf16, tag="pt", name="pt")
                        nc.scalar.activation(two_slabs(pt, c0, w),
                                             two_slabs(sc, c0, w), Exp)
                        if v >= 0:
                            nc.vector.tensor_mul(two_slabs(pt, c0, P),
                                                 two_slabs(pt, c0, P),
                                                 bcast2(tri_sb, P))
                        for hh in range(2):
                            h = 2 * db + hh
                            nc.tensor.matmul(
                                av[:, hh * 512 + c0:(hh + 1) * 512],
                                v_sb[t][:, h * (DK + 1):(h + 1) * (DK + 1)],
                                pt[:, hh * 512 + c0:(hh + 1) * 512],
                                start=(t == 0), stop=(t == nt - 1))
                    rcp = rcpp.tile([1, 1024], f32, tag="rcp", name="rcp")
                    nc.vector.reciprocal(rcp, av[DK:DK + 1, :])
                    rmat = rmatp.tile([DK, 1024], f32, tag="rmat", name="rmat")
                    nc.gpsimd.partition_broadcast(rmat, rcp, channels=DK)
                    for hh in range(2):
                        po = hh * DK
                        nc.vector.tensor_mul(
                            ot_sb[db][po:po + DK, cols],
                            av[0:DK, hh * 512:(hh + 1) * 512],
                            rmat[:, hh * 512:(hh + 1) * 512])

            def oproj_group(g):
                for m in range(4 * g, 4 * g + 4):
                    for nb in range(2):
                        ps = mm_ps.tile([P, 512], f32, tag="mm", name="mm")
                        for k in range(4):
                            nc.tensor.matmul(
                                ps, ot_sb[k][:, m * P:(m + 1) * P],
                                wo_sb[k][:, nb * 512:(nb + 1) * 512],
                                start=(k == 0), stop=(k == 3))
                        og = ostg.tile([P, 512], bf16, tag="og", name="og")
                        nc.vector.tensor_copy(og, ps)
                        nc.sync.dma_start(
                            out=out[m * P:(m + 1) * P,
                                    nb * 512:(nb + 1) * 512],
                            in_=og)

            # software-staged emission: keep PE fed with proj work while the
            # ACT-heavy attention of earlier groups drains
            proj_group(0)
            proj_group(1)
            for g in range(4):
                attn_group(g)
                if g + 2 < 4:
                    proj_group(g + 2)
                oproj_group(g)

        for _ in range(reps):
            emit_body()

    nc.compile()
    return nc


def _get_nc(reps=1):
    key = ("nc", reps)
    if key not in _CACHE:
        _CACHE[key] = _build(reps)
    return _CACHE[key]


def _prep_core_inputs(q_proj_weight, k_proj_weight, v_proj_weight,
                      o_proj_weight, in_features, token_positions):
    """Host-side sharding: returns the list of 8 per-core input dicts."""
    import ml_dtypes
    bf = ml_dtypes.bfloat16

    x = np.asarray(in_features, np.float32)
    wqf = np.asarray(q_proj_weight, np.float32)
    wkf = np.asarray(k_proj_weight, np.float32)
    wvf = np.asarray(v_proj_weight, np.float32)
    wof = np.asarray(o_proj_weight, np.float32)
    tp = np.asarray(token_positions).astype(np.float64)

    inv = 1.0 / (THETA ** (np.arange(HALF, dtype=np.float64) / HALF))
    fr = tp[:, None] * inv[None, :]                       # [S, 32]
    cosn = np.repeat(np.cos(fr), 2, axis=1).astype(bf)    # [S, 64]
    sg = np.tile(np.array([-1.0, 1.0]), HALF)[None, :]
    sinn = (np.repeat(np.sin(fr), 2, axis=1) * sg).astype(bf)

    kv = np.arange(P)[:, None]
    qc = np.arange(P)[None, :]
    tri = (qc >= kv).astype(bf)                           # [128, 128]

    identity = np.eye(P, dtype=bf)

    in_maps = []
    for c in range(N_CORES):
        b, hg = c // 2, c % 2
        rows = slice(hg * DOUT, (hg + 1) * DOUT)
        blob = np.empty(BLOB_N, dtype=bf)

        def put(name, arr):
            fl = np.ascontiguousarray(arr, dtype=bf).reshape(-1)
            blob[_OFF[name]:_OFF[name] + fl.size] = fl

        put("xT", x[b].T)
        put("wq", (wqf[rows] * SCALE).T)
        put("wk", wkf[rows].T)
        put("wv", wvf[rows].T)
        put("cos", cosn)
        put("sin", sinn)
        put("tri", tri)
        put("ident", identity)
        put("wo", wof[:, rows].T)
        in_maps.append({"blob": blob})
    return in_maps


def kernel(q_proj_weight, k_proj_weight, v_proj_weight, o_proj_weight,
           in_features, token_positions):
    from concourse.bass_utils import run_bass_kernel_spmd

    nc = _get_nc()
    in_maps = _prep_core_inputs(q_proj_weight, k_proj_weight, v_proj_weight,
                                o_proj_weight, in_features, token_positions)
    trace = bool(int(os.environ.get("KBENCH_TRACE", "0")))
    res = run_bass_kernel_spmd(nc, in_maps, list(range(N_CORES)), trace=trace)
    _CACHE["last_results"] = res
    if res.exec_time_ns is not None:
        _CACHE["exec_time_ns"] = res.exec_time_ns

    outp = np.empty((B, S, D_MODEL), np.float32)
    for b in range(B):
        outp[b] = (res.results[2 * b]["out"].astype(np.float32)
                   + res.results[2 * b + 1]["out"].astype(np.float32))
    return outp
